# revision 7
# baseline (speedup 1.0000x reference)
"""DocRED relation-extraction head on 8 Trainium2 NeuronCores.

Data-parallel over the batch axis: core b owns batch b's hidden_states slab
and its entity/pair indices; classifier weights are replicated.

Key algebraic restructuring vs the naive graph: instead of materializing
rel = concat(subj, obj) [P, 2H] and doing [P,2H]@[2H,H]@[H,97] per pair,
project the 32 entity representations first:
    eL1 = (rep @ W1) @ out_w   [E, 97]
    eL2 = (rep @ W2) @ out_w   [E, 97]
    logits[p] = eL1[head[p]] + eL2[tail[p]] + (dense_b @ out_w + out_b)
The pair gather becomes two tiny one-hot matmuls accumulated in PSUM.
This cuts the matmul work 32x and the device only ever reads the 128
gathered token rows of hidden_states (indirect DMA), dense_w, and out_w.
"""

import numpy as np
from contextlib import ExitStack

import concourse.bass as bass
import concourse.bacc as bacc
import concourse.tile as tile
import concourse.mybir as mybir
from concourse.bass_utils import run_bass_kernel_spmd

B, L, H, E, M, P, C = 8, 2048, 1024, 32, 4, 1024, 97
N_CORES = 8
HC = H // 128   # h-dim chunks (contraction of dense)
JC = H // 128   # j-dim chunks (output of dense / contraction of out proj)
PT = P // 128   # pair tiles

f32 = mybir.dt.float32
i32 = mybir.dt.int32

_CACHE = {}


def _build():
    nc = bacc.Bacc("TRN2", target_bir_lowering=False, debug=False)

    hs = nc.dram_tensor("hs", [L, H], f32, kind="ExternalInput").ap()
    pos = nc.dram_tensor("pos", [E * M, 1], i32, kind="ExternalInput").ap()
    headrep = nc.dram_tensor("headrep", [E, P], f32, kind="ExternalInput").ap()
    tailrep = nc.dram_tensor("tailrep", [E, P], f32, kind="ExternalInput").ap()
    iota_c = nc.dram_tensor("iota_c", [E, 1], f32, kind="ExternalInput").ap()
    onesblk = nc.dram_tensor("onesblk", [E * M, E], f32, kind="ExternalInput").ap()
    dw = nc.dram_tensor("dw", [2 * H, H], f32, kind="ExternalInput").ap()
    db = nc.dram_tensor("db", [H], f32, kind="ExternalInput").ap()
    ow = nc.dram_tensor("ow", [H, C], f32, kind="ExternalInput").ap()
    ob = nc.dram_tensor("ob", [1, C], f32, kind="ExternalInput").ap()
    out = nc.dram_tensor("out", [P, C], f32, kind="ExternalOutput").ap()

    with tile.TileContext(nc) as tc, ExitStack() as ctx:
        sb = ctx.enter_context(tc.tile_pool(name="sb", bufs=1))
        wpool = ctx.enter_context(tc.tile_pool(name="w", bufs=4))
        opool = ctx.enter_context(tc.tile_pool(name="o", bufs=2))
        # One accumulation group per PSUM bank at a time (start=True clears
        # has_written for the whole bank) -> single 8-slot pool, bank per slot.
        pspool = ctx.enter_context(tc.tile_pool(name="ps", bufs=8, space="PSUM"))

        # ---- small input loads
        sb_pos = sb.tile([E * M, 1], i32)
        nc.sync.dma_start(sb_pos[:], pos[:])
        sb_ones = sb.tile([E * M, E], f32)
        nc.sync.dma_start(sb_ones[:], onesblk[:])
        sb_iota = sb.tile([E, 1], f32)
        nc.sync.dma_start(sb_iota[:], iota_c[:])
        sb_hr = sb.tile([E, P], f32)
        nc.sync.dma_start(sb_hr[:], headrep[:])
        sb_tr = sb.tile([E, P], f32)
        nc.sync.dma_start(sb_tr[:], tailrep[:])
        sb_db = sb.tile([128, HC], f32)
        nc.sync.dma_start(sb_db[:], db.rearrange("(c p) -> p c", p=128))
        sb_ob = sb.tile([1, C], f32)
        nc.sync.dma_start(sb_ob[:], ob[:])
        sb_ow = sb.tile([128, JC * C], f32)
        nc.sync.dma_start(
            sb_ow[:].rearrange("p (j c) -> p j c", j=JC),
            ow.rearrange("(j p) c -> p j c", p=128),
        )

        # ---- gather the 128 mention rows of hidden_states
        sb_g = sb.tile([E * M, H], f32)
        nc.gpsimd.indirect_dma_start(
            out=sb_g[:],
            out_offset=None,
            in_=hs[:],
            in_offset=bass.IndirectOffsetOnAxis(ap=sb_pos[:, :1], axis=0),
        )

        # ---- stage A: entity_repT[h, e] = sum_m gathered[4e+m, h]
        # (mention-sum and transpose fused into 8 matmuls vs block-ones)
        sb_repT = sb.tile([128, HC * E], f32)
        for hc in range(HC):
            pa = pspool.tile([128, E], f32, tag="ps")
            nc.tensor.matmul(
                out=pa[:],
                lhsT=sb_g[:, hc * 128:(hc + 1) * 128],
                rhs=sb_ones[:],
                start=True,
                stop=True,
            )
            nc.vector.tensor_copy(out=sb_repT[:, hc * E:(hc + 1) * E], in_=pa[:])

        # ---- stage B: projT[j, e] = sum_h W[h, j] * repT[h, e]
        # Halves sequential; within a half, 8 j-chunk accumulation groups run
        # concurrently, each in its own PSUM bank.
        sb_projT = sb.tile([128, 2 * JC * E], f32)
        for half in range(2):
            ps_b = [pspool.tile([128, E], f32, tag="ps", name=f"ps_b{half}_{jc}")
                    for jc in range(JC)]
            for hc in range(HC):
                s = half * HC + hc
                wt = wpool.tile([128, H], f32, tag="wslab")
                nc.sync.dma_start(wt[:], dw[s * 128:(s + 1) * 128, :])
                for jc in range(JC):
                    nc.tensor.matmul(
                        out=ps_b[jc][:],
                        lhsT=wt[:, jc * 128:(jc + 1) * 128],
                        rhs=sb_repT[:, hc * E:(hc + 1) * E],
                        start=(hc == 0),
                        stop=(hc == HC - 1),
                    )
            for jc in range(JC):
                sl = (half * JC + jc) * E
                nc.vector.tensor_copy(out=sb_projT[:, sl:sl + E], in_=ps_b[jc][:])

        # ---- stage C: eL[e, c]; const row = dense_b @ out_w + out_b folded
        # into the eL1 accumulation group via a rank-1 matmul
        ps_cst = pspool.tile([1, C], f32, tag="ps")
        for jc in range(JC):
            nc.tensor.matmul(
                out=ps_cst[:],
                lhsT=sb_db[:, jc:jc + 1],
                rhs=sb_ow[:, jc * C:(jc + 1) * C],
                start=(jc == 0),
                stop=(jc == JC - 1),
            )
        sb_cst = sb.tile([1, C], f32)
        nc.vector.tensor_add(out=sb_cst[:], in0=ps_cst[:], in1=sb_ob[:])
        sb_one_row = sb.tile([1, E], f32)
        nc.vector.memset(sb_one_row[:], 1.0)

        ps_eL = pspool.tile([E, 2 * C], f32, tag="ps")
        for half in range(2):
            for jc in range(JC):
                nc.tensor.matmul(
                    out=ps_eL[:, half * C:(half + 1) * C],
                    lhsT=sb_projT[:, (half * JC + jc) * E:(half * JC + jc + 1) * E],
                    rhs=sb_ow[:, jc * C:(jc + 1) * C],
                    start=(jc == 0),
                    stop=(jc == JC - 1) and half == 1,
                )
            if half == 0:
                # close the eL1 group by adding the const row to every entity
                nc.tensor.matmul(
                    out=ps_eL[:, 0:C],
                    lhsT=sb_one_row[:],
                    rhs=sb_cst[:],
                    start=False,
                    stop=True,
                )
        sb_eL = sb.tile([E, 2 * C], f32)
        nc.vector.tensor_copy(out=sb_eL[:], in_=ps_eL[:])

        # ---- stage D: one-hot pair gather, logits[p] = eL1[head] + eL2[tail]
        sb_oh = sb.tile([E, P], f32)
        nc.vector.tensor_tensor(
            out=sb_oh[:],
            in0=sb_iota[:, :1].to_broadcast([E, P]),
            in1=sb_hr[:],
            op=mybir.AluOpType.is_equal,
        )
        sb_ot = sb.tile([E, P], f32)
        nc.vector.tensor_tensor(
            out=sb_ot[:],
            in0=sb_iota[:, :1].to_broadcast([E, P]),
            in1=sb_tr[:],
            op=mybir.AluOpType.is_equal,
        )
        for pt in range(PT):
            pl = pspool.tile([128, C], f32, tag="ps")
            nc.tensor.matmul(
                out=pl[:],
                lhsT=sb_oh[:, pt * 128:(pt + 1) * 128],
                rhs=sb_eL[:, :C],
                start=True,
                stop=False,
            )
            nc.tensor.matmul(
                out=pl[:],
                lhsT=sb_ot[:, pt * 128:(pt + 1) * 128],
                rhs=sb_eL[:, C:],
                start=False,
                stop=True,
            )
            ot = opool.tile([128, C], f32, tag="ot")
            nc.vector.tensor_copy(out=ot[:], in_=pl[:])
            nc.sync.dma_start(out[pt * 128:(pt + 1) * 128, :], ot[:])

    nc.compile()
    return nc


def get_compiled():
    if "nc" not in _CACHE:
        _CACHE["nc"] = _build()
    return _CACHE["nc"]


def make_in_maps(hidden_states, dense_w, dense_b, out_w, out_b,
                 entity_position_ids, head_tail_idxs):
    iota_c = np.arange(E, dtype=np.float32).reshape(E, 1)
    onesblk = np.repeat(np.eye(E, dtype=np.float32), M, axis=0)  # [E*M, E]
    dense_w = np.ascontiguousarray(dense_w, dtype=np.float32)
    out_w = np.ascontiguousarray(out_w, dtype=np.float32)
    dense_b = np.ascontiguousarray(dense_b, dtype=np.float32)
    ob = np.ascontiguousarray(out_b, dtype=np.float32).reshape(1, C)
    in_maps = []
    for b in range(B):
        ht = head_tail_idxs[b].astype(np.float32)  # [P, 2]
        in_maps.append({
            "hs": np.ascontiguousarray(hidden_states[b], dtype=np.float32),
            "pos": np.ascontiguousarray(
                entity_position_ids[b].reshape(E * M, 1).astype(np.int32)),
            "headrep": np.ascontiguousarray(
                np.broadcast_to(ht[None, :, 0], (E, P))),
            "tailrep": np.ascontiguousarray(
                np.broadcast_to(ht[None, :, 1], (E, P))),
            "iota_c": iota_c,
            "onesblk": onesblk,
            "dw": dense_w,
            "db": dense_b,
            "ow": out_w,
            "ob": ob,
        })
    return in_maps


def kernel(hidden_states, dense_w, dense_b, out_w, out_b,
           entity_position_ids, head_tail_idxs, _trace=False, _trace_kwargs=None):
    nc = get_compiled()
    in_maps = make_in_maps(hidden_states, dense_w, dense_b, out_w, out_b,
                           entity_position_ids, head_tail_idxs)
    res = run_bass_kernel_spmd(
        nc, in_maps, core_ids=list(range(N_CORES)),
        trace=_trace, **(_trace_kwargs or {}),
    )
    outp = np.concatenate([res.results[i]["out"] for i in range(N_CORES)], axis=0)
    if _trace:
        return outp, res
    return outp


# revision 8
# speedup vs baseline: 1.2420x; 1.2420x over previous
"""DocRED relation-extraction head on 8 Trainium2 NeuronCores.

Data-parallel over the batch axis: core b owns batch b's hidden_states slab
and its entity/pair indices; classifier weights are replicated.

Key algebraic restructuring vs the naive graph: instead of materializing
rel = concat(subj, obj) [P, 2H] and doing [P,2H]@[2H,H]@[H,97] per pair,
project the 32 entity representations first:
    eL1 = (rep @ W1) @ out_w  (+ dense_b @ out_w + out_b folded in)
    eL2 = (rep @ W2) @ out_w
    logits[p] = eL1[head[p]] + eL2[tail[p]] + const
The pair gather becomes two tiny one-hot matmuls accumulated in PSUM.
This cuts the matmul work 32x, and the device only ever reads the 128
gathered token rows of hidden_states (indirect DMA), dense_w, and out_w.

fp32 matmul on trn2 is a 2-pass (hi/lo) operation, so W is streamed as the
MOVING operand in N=512 chunks (few big matmuls); the resulting proj [32, j]
is flipped to projT [j, 32] with DVE 32x32 stream-transposes, keeping PE free.
dense_b rides along as lhsT column 32 in the out_w projection; the resulting
const row is applied per-pair through an all-ones row 32 in the head one-hot.
"""

import numpy as np
from contextlib import ExitStack

import concourse.bass as bass
import concourse.bacc as bacc
import concourse.tile as tile
import concourse.mybir as mybir
from concourse.bass_utils import run_bass_kernel_spmd

B, L, H, E, M, P, C = 8, 2048, 1024, 32, 4, 1024, 97
N_CORES = 8
HC = H // 128   # h-dim chunks (contraction of dense)
JC = H // 128   # j-dim chunks (output of dense / contraction of out proj)
PT = P // 128   # pair tiles
SLOT = E + 1    # projT slot width: 32 cols projT + 1 col dense_b chunk

f32 = mybir.dt.float32
i32 = mybir.dt.int32

_CACHE = {}


def _build():
    nc = bacc.Bacc("TRN2", target_bir_lowering=False, debug=False)

    hs = nc.dram_tensor("hs", [L, H], f32, kind="ExternalInput").ap()
    pos = nc.dram_tensor("pos", [E * M, 1], i32, kind="ExternalInput").ap()
    headrep = nc.dram_tensor("headrep", [E, P], f32, kind="ExternalInput").ap()
    tailrep = nc.dram_tensor("tailrep", [E, P], f32, kind="ExternalInput").ap()
    iota_c = nc.dram_tensor("iota_c", [E, 1], f32, kind="ExternalInput").ap()
    onesblk = nc.dram_tensor("onesblk", [E * M, E], f32, kind="ExternalInput").ap()
    dw = nc.dram_tensor("dw", [2 * H, H], f32, kind="ExternalInput").ap()
    db = nc.dram_tensor("db", [H], f32, kind="ExternalInput").ap()
    ow = nc.dram_tensor("ow", [H, C], f32, kind="ExternalInput").ap()
    ob = nc.dram_tensor("ob", [1, C], f32, kind="ExternalInput").ap()
    out = nc.dram_tensor("out", [P, C], f32, kind="ExternalOutput").ap()

    with tile.TileContext(nc) as tc, ExitStack() as ctx:
        sb = ctx.enter_context(tc.tile_pool(name="sb", bufs=1))
        wpool = ctx.enter_context(tc.tile_pool(name="w", bufs=6))
        opool = ctx.enter_context(tc.tile_pool(name="o", bufs=2))
        # One accumulation group per PSUM bank at a time (start=True clears
        # has_written for the whole bank) -> single 8-slot pool, bank per slot.
        pspool = ctx.enter_context(tc.tile_pool(name="ps", bufs=8, space="PSUM"))

        # ---- small input loads
        sb_pos = sb.tile([E * M, 1], i32)
        nc.sync.dma_start(sb_pos[:], pos[:])
        sb_ones = sb.tile([E * M, E], f32)
        nc.sync.dma_start(sb_ones[:], onesblk[:])
        sb_iota = sb.tile([E, 1], f32)
        nc.sync.dma_start(sb_iota[:], iota_c[:])
        sb_hr = sb.tile([E, P], f32)
        nc.sync.dma_start(sb_hr[:], headrep[:])
        sb_tr = sb.tile([E, P], f32)
        nc.sync.dma_start(sb_tr[:], tailrep[:])
        sb_db = sb.tile([128, HC], f32)
        nc.sync.dma_start(sb_db[:], db.rearrange("(c p) -> p c", p=128))
        sb_ob = sb.tile([1, C], f32)
        nc.sync.dma_start(sb_ob[:], ob[:])
        sb_ow = sb.tile([128, JC * C], f32)
        nc.sync.dma_start(
            sb_ow[:].rearrange("p (j c) -> p j c", j=JC),
            ow.rearrange("(j p) c -> p j c", p=128),
        )

        # ---- gather the 128 mention rows of hidden_states
        sb_g = sb.tile([E * M, H], f32)
        nc.gpsimd.indirect_dma_start(
            out=sb_g[:],
            out_offset=None,
            in_=hs[:],
            in_offset=bass.IndirectOffsetOnAxis(ap=sb_pos[:, :1], axis=0),
        )

        # ---- stage A: entity_repT[h, e] = sum_m gathered[4e+m, h]
        # (mention-sum and transpose fused into 8 matmuls vs block-ones)
        sb_repT = sb.tile([128, HC * E], f32)
        for hc in range(HC):
            pa = pspool.tile([128, E], f32, tag="ps", name=f"pa{hc}")
            nc.tensor.matmul(
                out=pa[:],
                lhsT=sb_g[:, hc * 128:(hc + 1) * 128],
                rhs=sb_ones[:],
                start=True,
                stop=True,
            )
            nc.vector.tensor_copy(out=sb_repT[:, hc * E:(hc + 1) * E], in_=pa[:])

        # ---- stage B: proj[e, j'] = sum_h repT[h, e] * W[h, j'] for the
        # concatenated j' = (half, j). repT chunk is the stationary operand,
        # W streams through in N=512 chunks; 4 bank accumulators.
        # proj is then flipped to projT via DVE 32x32 stream-transposes, with
        # the dense_b chunk appended as column 32 of each slot.
        sb_projT = sb.tile([128, 2 * JC * SLOT], f32)
        ps_p = [pspool.tile([32, 512], f32, tag="ps", name=f"ps_p{q}")
                for q in range(4)]
        for hc in range(HC):
            wt1 = wpool.tile([128, H], f32, tag="wslab", name=f"wt1_{hc}")
            nc.sync.dma_start(wt1[:], dw[hc * 128:(hc + 1) * 128, :])
            wt2 = wpool.tile([128, H], f32, tag="wslab", name=f"wt2_{hc}")
            nc.sync.dma_start(wt2[:], dw[(HC + hc) * 128:(HC + hc + 1) * 128, :])
            for q, wt in ((0, wt1), (1, wt1), (2, wt2), (3, wt2)):
                nc.tensor.matmul(
                    out=ps_p[q][:],
                    lhsT=sb_repT[:, hc * E:(hc + 1) * E],
                    rhs=wt[:, (q % 2) * 512:(q % 2 + 1) * 512],
                    start=(hc == 0),
                    stop=(hc == HC - 1),
                )
        # transpose proj -> projT (DVE stream-transpose works on 32x32 blocks)
        for half in range(2):
            for jc in range(JC):
                slot = (half * JC + jc) * SLOT
                for bl in range(4):
                    j0 = jc * 128 + bl * 32          # j offset within the half
                    q = half * 2 + j0 // 512
                    nc.vector.transpose(
                        out=sb_projT[bl * 32:(bl + 1) * 32, slot:slot + E],
                        in_=ps_p[q][:, j0 % 512:j0 % 512 + 32],
                    )
                # dense_b chunk rides along as lhsT column 32 (half 0 only)
                if half == 0:
                    nc.vector.tensor_copy(
                        out=sb_projT[:, slot + E:slot + E + 1],
                        in_=sb_db[:, jc:jc + 1],
                    )

        # ---- stage C: eL1' [33, 97] (rows 0-31 eL1, row 32 dense_b @ out_w),
        # eL2 [32, 97]; accumulate over j chunks.
        ps_eL1 = pspool.tile([SLOT, C], f32, tag="ps")
        ps_eL2 = pspool.tile([E, C], f32, tag="ps")
        for jc in range(JC):
            nc.tensor.matmul(
                out=ps_eL1[:],
                lhsT=sb_projT[:, jc * SLOT:jc * SLOT + SLOT],
                rhs=sb_ow[:, jc * C:(jc + 1) * C],
                start=(jc == 0),
                stop=(jc == JC - 1),
            )
        for jc in range(JC):
            nc.tensor.matmul(
                out=ps_eL2[:],
                lhsT=sb_projT[:, (JC + jc) * SLOT:(JC + jc) * SLOT + E],
                rhs=sb_ow[:, jc * C:(jc + 1) * C],
                start=(jc == 0),
                stop=(jc == JC - 1),
            )
        sb_eL = sb.tile([SLOT, 2 * C], f32)
        nc.vector.tensor_copy(out=sb_eL[:, :C], in_=ps_eL1[:])
        nc.vector.tensor_copy(out=sb_eL[:E, C:], in_=ps_eL2[:])
        # row 32 of eL1' = dense_b @ out_w; add out_b to finish the const row
        nc.vector.tensor_add(
            out=sb_eL[E:E + 1, :C], in0=ps_eL1[E:E + 1, :], in1=sb_ob[:])

        # ---- stage D: one-hot pair gather,
        # logits[p] = eL1[head] + const + eL2[tail] via PSUM accumulation.
        sb_oh = sb.tile([SLOT, P], f32)
        nc.vector.tensor_tensor(
            out=sb_oh[:E, :],
            in0=sb_iota[:, :1].to_broadcast([E, P]),
            in1=sb_hr[:],
            op=mybir.AluOpType.is_equal,
        )
        nc.vector.memset(sb_oh[E:E + 1, :], 1.0)  # row 32: add const to all p
        sb_ot = sb.tile([E, P], f32)
        nc.vector.tensor_tensor(
            out=sb_ot[:],
            in0=sb_iota[:, :1].to_broadcast([E, P]),
            in1=sb_tr[:],
            op=mybir.AluOpType.is_equal,
        )
        for pt in range(PT):
            pl = pspool.tile([128, C], f32, tag="ps", name=f"pl{pt}")
            nc.tensor.matmul(
                out=pl[:],
                lhsT=sb_oh[:, pt * 128:(pt + 1) * 128],
                rhs=sb_eL[:, :C],
                start=True,
                stop=False,
            )
            nc.tensor.matmul(
                out=pl[:],
                lhsT=sb_ot[:, pt * 128:(pt + 1) * 128],
                rhs=sb_eL[:E, C:],
                start=False,
                stop=True,
            )
            ot = opool.tile([128, C], f32, tag="ot", name=f"ot{pt}")
            nc.vector.tensor_copy(out=ot[:], in_=pl[:])
            nc.sync.dma_start(out[pt * 128:(pt + 1) * 128, :], ot[:])

    nc.compile()
    return nc


def get_compiled():
    if "nc" not in _CACHE:
        _CACHE["nc"] = _build()
    return _CACHE["nc"]


def make_in_maps(hidden_states, dense_w, dense_b, out_w, out_b,
                 entity_position_ids, head_tail_idxs):
    iota_c = np.arange(E, dtype=np.float32).reshape(E, 1)
    onesblk = np.repeat(np.eye(E, dtype=np.float32), M, axis=0)  # [E*M, E]
    dense_w = np.ascontiguousarray(dense_w, dtype=np.float32)
    out_w = np.ascontiguousarray(out_w, dtype=np.float32)
    dense_b = np.ascontiguousarray(dense_b, dtype=np.float32)
    ob = np.ascontiguousarray(out_b, dtype=np.float32).reshape(1, C)
    in_maps = []
    for b in range(B):
        ht = head_tail_idxs[b].astype(np.float32)  # [P, 2]
        in_maps.append({
            "hs": np.ascontiguousarray(hidden_states[b], dtype=np.float32),
            "pos": np.ascontiguousarray(
                entity_position_ids[b].reshape(E * M, 1).astype(np.int32)),
            "headrep": np.ascontiguousarray(
                np.broadcast_to(ht[None, :, 0], (E, P))),
            "tailrep": np.ascontiguousarray(
                np.broadcast_to(ht[None, :, 1], (E, P))),
            "iota_c": iota_c,
            "onesblk": onesblk,
            "dw": dense_w,
            "db": dense_b,
            "ow": out_w,
            "ob": ob,
        })
    return in_maps


def kernel(hidden_states, dense_w, dense_b, out_w, out_b,
           entity_position_ids, head_tail_idxs, _trace=False, _trace_kwargs=None):
    nc = get_compiled()
    in_maps = make_in_maps(hidden_states, dense_w, dense_b, out_w, out_b,
                           entity_position_ids, head_tail_idxs)
    res = run_bass_kernel_spmd(
        nc, in_maps, core_ids=list(range(N_CORES)),
        trace=_trace, **(_trace_kwargs or {}),
    )
    outp = np.concatenate([res.results[i]["out"] for i in range(N_CORES)], axis=0)
    if _trace:
        return outp, res
    return outp


# revision 14
# speedup vs baseline: 1.3753x; 1.1073x over previous
"""DocRED relation-extraction head on 8 Trainium2 NeuronCores.

Data-parallel over the batch axis: core b owns batch b's hidden_states slab
and its entity/pair indices; classifier weights are replicated.

Algorithm (per core, one batch):
    rep[e]  = sum_m hidden_states[entity_position_ids[e, m]]      (indirect DMA
              gather of 128 rows + mention-sum matmul, transposed: repT [h, e])
    projT   = (rep @ [W1 | W2]).T        W streamed as the moving operand,
              proj flipped back with DVE 32x32 stream-transposes
    eL1'    = [projT1 | dense_b].T @ out_w   [33, 97]  (row 32 = const row)
    eL2     = projT2.T @ out_w               [32, 97]
    logits[p] = eL1'[head[p]] + const + eL2[tail[p]]   via one K=65-stacked
              one-hot matmul per 128-pair tile (PSUM accumulation).

Matmul inputs use float32r (single-pass reduced-precision fp32, ~1e-4 rel) —
plain fp32 matmul on trn2 lowers to 2 ISA passes and doubles PE time.
Set MM_DT = f32 below to switch back to exact fp32.
"""

import numpy as np
from contextlib import ExitStack

import concourse.bass as bass
import concourse.bacc as bacc
import concourse.tile as tile
import concourse.mybir as mybir
from concourse.bass_utils import run_bass_kernel_spmd

B, L, H, E, M, P, C = 8, 2048, 1024, 32, 4, 1024, 97
N_CORES = 8
HC = H // 128   # h-dim chunks (contraction of dense)
JC = H // 128   # j-dim chunks (output of dense / contraction of out proj)
PT = P // 128   # pair tiles
SLOT = E + 1    # projT slot width: 32 cols projT + 1 col dense_b chunk

f32 = mybir.dt.float32
f32r = mybir.dt.float32r
i32 = mybir.dt.int32

MM_DT = f32r    # dtype of matmul input tiles (f32r: 1-pass PE, ~1e-4 rel err)

CP = C + 1                # class dim padded to 98: f32r needs an even
                          # moving dim; the pad column is zero end to end

# constant-blob column layout
ONES0 = 1                 # [128, 32] mention-sum block-ones
DB0 = ONES0 + E           # [128, 8] dense_b chunks
OW0 = DB0 + HC            # [128, 8*98] out_w chunks (zero pad col each)
IOTA0 = OW0 + JC * CP     # [32, 1] iota column
OB0 = IOTA0 + 1           # [1, 98] out_b on row 0 (zero padded)
BLOBW = OB0 + CP

_CACHE = {}


def _build():
    nc = bacc.Bacc("TRN2", target_bir_lowering=False, debug=False)

    hs = nc.dram_tensor("hs", [L, H], MM_DT, kind="ExternalInput").ap()
    pos = nc.dram_tensor("pos", [E * M, 1], i32, kind="ExternalInput").ap()
    blob = nc.dram_tensor("blob", [128, BLOBW], MM_DT, kind="ExternalInput").ap()
    headrep = nc.dram_tensor("headrep", [E, P], MM_DT, kind="ExternalInput").ap()
    tailrep = nc.dram_tensor("tailrep", [E, P], MM_DT, kind="ExternalInput").ap()
    dw = nc.dram_tensor("dw", [2 * H, H], MM_DT, kind="ExternalInput").ap()
    out = nc.dram_tensor("out", [P, C], f32, kind="ExternalOutput").ap()

    with tile.TileContext(nc) as tc, ExitStack() as ctx:
        sb = ctx.enter_context(tc.tile_pool(name="sb", bufs=1))
        wpool = ctx.enter_context(tc.tile_pool(name="w", bufs=8))
        opool = ctx.enter_context(tc.tile_pool(name="o", bufs=2))
        # One accumulation group per PSUM bank at a time (start=True clears
        # has_written for the whole bank) -> single 8-slot pool, bank per slot.
        pspool = ctx.enter_context(tc.tile_pool(name="ps", bufs=8, space="PSUM"))

        # ---- latency-critical small inputs on the scalar HWDGE ring
        sb_pos = sb.tile([E * M, 1], i32)
        nc.scalar.dma_start(sb_pos[:], pos[:])
        sb_blob = sb.tile([128, BLOBW], MM_DT)
        nc.scalar.dma_start(sb_blob[:], blob[:])
        sb_hr = sb.tile([E, P], MM_DT)
        nc.scalar.dma_start(sb_hr[:], headrep[:])
        sb_tr = sb.tile([E, P], MM_DT)
        nc.scalar.dma_start(sb_tr[:], tailrep[:])

        # ---- gather the 128 mention rows of hidden_states
        sb_g = sb.tile([E * M, H], MM_DT)
        nc.gpsimd.indirect_dma_start(
            out=sb_g[:],
            out_offset=None,
            in_=hs[:],
            in_offset=bass.IndirectOffsetOnAxis(ap=sb_pos[:, :1], axis=0),
        )

        # ---- stage A: entity_repT[h, e] = sum_m gathered[4e+m, h]
        # (mention-sum and transpose fused into 8 matmuls vs block-ones)
        sb_repT = sb.tile([128, HC * E], MM_DT)
        for hc in range(HC):
            pa = pspool.tile([128, E], f32, tag="ps", name=f"pa{hc}")
            nc.tensor.matmul(
                out=pa[:],
                lhsT=sb_g[:, hc * 128:(hc + 1) * 128],
                rhs=sb_blob[:, ONES0:ONES0 + E],
                start=True,
                stop=True,
            )
            nc.vector.tensor_copy(out=sb_repT[:, hc * E:(hc + 1) * E], in_=pa[:])

        # ---- stage B + C fused over two j-blocks of 512 columns.
        # Per block: stream 16 W sub-slabs [128, 512] (moving operand) through
        # repT (stationary) into two bank accumulators (proj1/proj2 for this
        # j-range), flip to projT via DVE 32x32 stream-transposes, then
        # immediately fold this block's j-chunks into the eL accumulators.
        sb_projT = sb.tile([128, 2 * JC * SLOT], MM_DT)
        # f32 staging for the stream-transposes (the ISA has no f32r variant);
        # cast-copied into the f32r projT right after.
        sb_projS = sb.tile([128, 2 * JC * SLOT], f32)
        ps_eL1 = pspool.tile([SLOT, CP], f32, tag="ps")
        ps_eL2 = pspool.tile([E, CP], f32, tag="ps")
        for nb in range(2):
            ps_blk = [pspool.tile([E, 512], f32, tag="ps", name=f"ps_blk{nb}_{h}")
                      for h in range(2)]
            for hc2 in range(2 * HC):
                half, hc = divmod(hc2, HC)
                wt = wpool.tile([128, 512], MM_DT, tag="wslab",
                                name=f"wt{nb}_{hc2}")
                eng = nc.sync if hc2 % 2 == 0 else nc.scalar
                eng.dma_start(
                    wt[:],
                    dw[(half * HC + hc) * 128:(half * HC + hc + 1) * 128,
                       nb * 512:(nb + 1) * 512],
                )
                nc.tensor.matmul(
                    out=ps_blk[half][:],
                    lhsT=sb_repT[:, hc * E:(hc + 1) * E],
                    rhs=wt[:],
                    start=(hc == 0),
                    stop=(hc == HC - 1),
                )
            for half in range(2):
                for jl in range(4):          # j-chunks within this block
                    jc = nb * 4 + jl
                    slot = (half * JC + jc) * SLOT
                    for bl in range(4):
                        nc.vector.transpose(
                            out=sb_projS[bl * 32:(bl + 1) * 32, slot:slot + E],
                            in_=ps_blk[half][:, jl * 128 + bl * 32:
                                             jl * 128 + bl * 32 + 32],
                        )
                    if half == 0:
                        # dense_b chunk rides along as lhsT column 32
                        nc.vector.tensor_copy(
                            out=sb_projS[:, slot + E:slot + E + 1],
                            in_=sb_blob[:, DB0 + jc:DB0 + jc + 1],
                        )
                # cast the f32 staging into the f32r lhsT for this
                # (block, half) range of slots
                s0 = (half * JC + nb * 4) * SLOT
                s1 = s0 + 4 * SLOT
                nc.vector.tensor_copy(
                    out=sb_projT[:, s0:s1], in_=sb_projS[:, s0:s1])
            for half in range(2):
                for jl in range(4):
                    jc = nb * 4 + jl
                    slot = (half * JC + jc) * SLOT
                    if half == 0:
                        nc.tensor.matmul(
                            out=ps_eL1[:],
                            lhsT=sb_projT[:, slot:slot + SLOT],
                            rhs=sb_blob[:, OW0 + jc * CP:OW0 + (jc + 1) * CP],
                            start=(jc == 0),
                            stop=(jc == JC - 1),
                        )
                    else:
                        nc.tensor.matmul(
                            out=ps_eL2[:],
                            lhsT=sb_projT[:, slot:slot + E],
                            rhs=sb_blob[:, OW0 + jc * CP:OW0 + (jc + 1) * CP],
                            start=(jc == 0),
                            stop=(jc == JC - 1),
                        )

        # ---- eL stack [65, 97] (partition offsets must be 32-aligned):
        # rows 0-31 = eL1, rows 32-63 = eL2, row 64 = dense_b @ out_w + out_b.
        sb_eL = sb.tile([2 * E + 1, CP], MM_DT)
        nc.vector.tensor_copy(out=sb_eL[:E, :], in_=ps_eL1[:E, :])
        nc.vector.tensor_copy(out=sb_eL[E:2 * E, :], in_=ps_eL2[:])
        nc.vector.tensor_add(
            out=sb_eL[2 * E:2 * E + 1, :], in0=ps_eL1[E:E + 1, :],
            in1=sb_blob[:1, OB0:OB0 + CP])

        # ---- stage D: stacked one-hot pair gather.
        # K rows 0-31: head one-hot; rows 32-63: tail; row 64: ones (const).
        sb_oh = sb.tile([2 * E + 1, P], MM_DT)
        nc.vector.tensor_tensor(
            out=sb_oh[:E, :],
            in0=sb_blob[:E, IOTA0:IOTA0 + 1].to_broadcast([E, P]),
            in1=sb_hr[:],
            op=mybir.AluOpType.is_equal,
        )
        nc.vector.tensor_tensor(
            out=sb_oh[E:2 * E, :],
            in0=sb_blob[:E, IOTA0:IOTA0 + 1].to_broadcast([E, P]),
            in1=sb_tr[:],
            op=mybir.AluOpType.is_equal,
        )
        # all-ones row via x==x (memset on an f32r tile fails the ISA check)
        nc.vector.tensor_tensor(
            out=sb_oh[2 * E:2 * E + 1, :],
            in0=sb_blob[:1, IOTA0:IOTA0 + 1].to_broadcast([1, P]),
            in1=sb_blob[:1, IOTA0:IOTA0 + 1].to_broadcast([1, P]),
            op=mybir.AluOpType.is_equal,
        )
        for pt in range(PT):
            pl = pspool.tile([128, CP], f32, tag="ps", name=f"pl{pt}")
            nc.tensor.matmul(
                out=pl[:],
                lhsT=sb_oh[:, pt * 128:(pt + 1) * 128],
                rhs=sb_eL[:],
                start=True,
                stop=True,
            )
            ot = opool.tile([128, C], f32, tag="ot", name=f"ot{pt}")
            nc.vector.tensor_copy(out=ot[:], in_=pl[:, :C])
            nc.sync.dma_start(out[pt * 128:(pt + 1) * 128, :], ot[:])

    nc.compile()
    return nc


def get_compiled():
    if "nc" not in _CACHE:
        _CACHE["nc"] = _build()
    return _CACHE["nc"]


def make_in_maps(hidden_states, dense_w, dense_b, out_w, out_b,
                 entity_position_ids, head_tail_idxs):
    blob = np.zeros((128, BLOBW), np.float32)
    blob[:, ONES0:ONES0 + E] = np.repeat(np.eye(E, dtype=np.float32), M, axis=0)
    blob[:, DB0:DB0 + HC] = np.asarray(dense_b, np.float32).reshape(HC, 128).T
    owp = np.zeros((H, CP), np.float32)
    owp[:, :C] = np.asarray(out_w, np.float32)
    blob[:, OW0:OW0 + JC * CP] = (
        owp.reshape(JC, 128, CP).transpose(1, 0, 2).reshape(128, JC * CP))
    blob[:E, IOTA0] = np.arange(E, dtype=np.float32)
    blob[0, OB0:OB0 + C] = np.asarray(out_b, np.float32)  # col 97 stays 0
    dense_w = np.ascontiguousarray(dense_w, dtype=np.float32)
    in_maps = []
    for b in range(B):
        ht = head_tail_idxs[b].astype(np.float32)  # [P, 2]
        in_maps.append({
            "hs": np.ascontiguousarray(hidden_states[b], dtype=np.float32),
            "pos": np.ascontiguousarray(
                entity_position_ids[b].reshape(E * M, 1).astype(np.int32)),
            "blob": blob,
            "headrep": np.ascontiguousarray(
                np.broadcast_to(ht[None, :, 0], (E, P))),
            "tailrep": np.ascontiguousarray(
                np.broadcast_to(ht[None, :, 1], (E, P))),
            "dw": dense_w,
        })
    return in_maps


def kernel(hidden_states, dense_w, dense_b, out_w, out_b,
           entity_position_ids, head_tail_idxs, _trace=False, _trace_kwargs=None):
    nc = get_compiled()
    in_maps = make_in_maps(hidden_states, dense_w, dense_b, out_w, out_b,
                           entity_position_ids, head_tail_idxs)
    res = run_bass_kernel_spmd(
        nc, in_maps, core_ids=list(range(N_CORES)),
        trace=_trace, **(_trace_kwargs or {}),
    )
    outp = np.concatenate([res.results[i]["out"] for i in range(N_CORES)], axis=0)
    if _trace:
        return outp, res
    return outp
